# revision 37
# baseline (speedup 1.0000x reference)
"""KoLeo-loss kernel, 8 NeuronCores — hybrid AllGather + HBM streaming, v4.

v4 vs baseline:
  - AllGathers trigger ~12us in (vs ~36us): gpsimd's queue carries only
    [stream loads, agin DMAs, AG triggers, scatters]; identity/selector
    constants are host-fed instead of gpsimd-built.
  - Own queries live in 4 per-chunk tiles QTc[qc] so the agin reads of
    chunk 0/1 don't create whole-tile WAR stalls against chunk 2/3
    transposes.
  - PE transposes pair (v,t=0),(v,t=1) into one 2-bank PSUM tile so the
    PSUM->SBUF copies are 40x[128,256] instead of 80x[128,128].
  - Streamed x loads split across gpsimd+sync DMA queues.
  - No warm-up transposes.

(XBAR dma_start_transpose was tried and reverted: the tile framework
serializes DMA transposes against in-flight collectives, which blocks
the issuing engine's queue for the whole AG window. Shared-HBM agout
was tried and reverted: pair cores corrupt each other's RDH scratch.)

Groups g = 2q+h hold sub-chunk q of ranks 4h..4h+3 at band (r%4)*128.
q in {0,1} arrive by 2 chunked AllGathers; q in {2,3} streamed from HBM
and re-normalized locally. Diagonal of core r for query sub-chunk mc
lives in group 2*mc+(r>=4), column (r%4)*128+p: injected via host-fed
selector (lhsT) x band pattern (rhs) bf16 matmul.

Math: fp8e4 keys scaled 32/||x||, t-paired DoubleRow Gram = 1024*cos,
row max M -> loss_i = -0.5*ln(2 - M/512); host sums 8 partials.
"""

import sys
from contextlib import ExitStack

import numpy as np

sys.path.insert(0, "/opt/trn_rl_repo")

import concourse.mybir as mybir
import concourse.tile as tile
from concourse import bacc, bass_utils

F32 = mybir.dt.float32
BF16 = mybir.dt.bfloat16
F8 = mybir.dt.float8e4
AF = mybir.ActivationFunctionType
DR = mybir.MatmulPerfMode.DoubleRow

B, V, D = 4096, 2, 1024
NCORES = 8
MB = B // NCORES          # 512 own rows per core
NQ = MB // 128            # 4 own sub-chunks
NG = 8
T = 2
EPS = 1e-8
MASKV = -8192.0
AGQ = (0, 1)              # sub-chunks delivered by AllGather
STQ = (2, 3)              # sub-chunks streamed from HBM


def build():
    nc = bacc.Bacc("TRN2", debug=False, num_devices=NCORES)
    x_d = nc.dram_tensor("x", [B, V, D], F32, kind="ExternalInput").ap()
    xq_d = nc.dram_tensor("xq", [MB, V, D], F32, kind="ExternalInput").ap()
    band_d = nc.dram_tensor("bandpat", [128, 512], F32, kind="ExternalInput").ap()
    hsel_d = nc.dram_tensor("hseli", [128, 2, 128], F32, kind="ExternalInput").ap()
    out_d = nc.dram_tensor("out", [128, V * (MB // 128)], F32, kind="ExternalOutput").ap()

    with ExitStack() as ctx:
        tc = ctx.enter_context(tile.TileContext(nc))
        const = ctx.enter_context(tc.tile_pool(name="const", bufs=1))
        xpool = ctx.enter_context(tc.tile_pool(name="xpool", bufs=8))
        xqpool = ctx.enter_context(tc.tile_pool(name="xqpool", bufs=4))
        ypool = ctx.enter_context(tc.tile_pool(name="ypool", bufs=3))
        sqpool = ctx.enter_context(tc.tile_pool(name="sqpool", bufs=2))
        sspool = ctx.enter_context(tc.tile_pool(name="sspool", bufs=3))
        accp = ctx.enter_context(tc.tile_pool(name="accp", bufs=5, space="PSUM"))
        trp = ctx.enter_context(tc.tile_pool(name="trp", bufs=1, space="PSUM"))
        smallp = ctx.enter_context(tc.tile_pool(name="smallp", bufs=1, space="PSUM"))
        dram = ctx.enter_context(tc.tile_pool(name="dram", bufs=1, space="DRAM"))

        # ---- constants (no gpsimd work: keep its queue free for AG) ----
        twob = const.tile([128, 1], F32, name="twob")
        nc.vector.memset(twob[:], 2.0)
        bandF = const.tile([128, 512], F32, name="bandF")
        nc.scalar.dma_start(bandF[:], band_d)
        bandB = const.tile([128, 512], BF16, name="bandB")
        nc.vector.tensor_copy(bandB[:], bandF[:])
        hsF = const.tile([128, 2, 128], F32, name="hsF")
        nc.scalar.dma_start(hsF[:], hsel_d)
        hselI = const.tile([128, 2, 128], BF16, name="hselI")
        nc.vector.tensor_copy(hselI[:], hsF[:])
        # identity built on gpsimd (first thing on its queue, ~1us;
        # host-fed identF lands too late behind the streamed-load pile)
        identF = const.tile([128, 128], F32, name="identF")
        nc.gpsimd.memset(identF[:], 0.0)
        nc.gpsimd.affine_select(
            out=identF[:], in_=identF[:], compare_op=mybir.AluOpType.not_equal,
            fill=1.0, base=0, pattern=[[-1, 128]], channel_multiplier=1)

        # ---- persistent buffers ----
        QTc = [const.tile([128, V, T, 128], F32, name=f"QT{qc}")
               for qc in range(NQ)]
        YTg = [const.tile([128, V, T, 512], F32, name=f"YT{g}")
               for g in range(NG)]
        mxs = const.tile([128, NG, V * NQ], F32, name="mxs")

        agin = [dram.tile([128, V, T, 128], F32, name=f"agin{q}") for q in AGQ]
        agout = [dram.tile([NCORES, 128, V, T, 128], F32, name=f"agout{q}")
                 for q in AGQ]

        def norm_quant(xt):
            """L2-normalize both views of a [128, V, D] f32 chunk, pack as
            fp8e4 scaled 32/||x|| into a [128, V, T, 128] f32-container tile."""
            ss = sspool.tile([128, V], F32, tag="ss", name="ss")
            sq = sqpool.tile([128, D], BF16, tag="sq", name="sq")
            for v in range(V):
                nc.scalar.activation(
                    sq[:], xt[:, v, :], AF.Square, accum_out=ss[:, v : v + 1])
            # (no +EPS: 1e-8 is below fp32 ulp of ||x||^2 ~ 1024 — the
            # reference's add rounds away bit-exactly for this data)
            rec = sspool.tile([128, V], F32, tag="rec", name="rec")
            nc.vector.reciprocal(rec[:], ss[:])
            rs = sspool.tile([128, V], F32, tag="rs", name="rs")
            nc.scalar.activation(rs[:], rec[:], AF.Sqrt, scale=1024.0)
            ypk = ypool.tile([128, V, T, 128], F32, tag="ypk", name="ypk")
            yp8 = ypk.bitcast(F8)
            nc.vector.tensor_scalar_mul(
                yp8[:, 0].rearrange("p t k -> p (t k)"), xt[:, 0, :],
                rs[:, 0:1])
            nc.vector.tensor_scalar_mul(
                yp8[:, 1].rearrange("p t k -> p (t k)"), xt[:, 1, :],
                rs[:, 1:2])
            return ypk, rs

        def transpose_into(ypk, dest):
            """PE-transpose ypk's 4 (v,t) f32-container tiles into one
            2-bank PSUM tile; a single strided DVE copy moves the whole
            chunk into dest (a [128, V, T, 128] f32 view). One copy per
            chunk instead of two: fewer instructions and semaphores (the
            NEFF epilogue walks every semaphore serially)."""
            tp4 = trp.tile([128, V, T, 128], F32, tag="tp", name="tp")
            for v in range(V):
                for t in range(T):
                    nc.tensor.transpose(tp4[:, v, t], ypk[:, v, t], identF[:])
            nc.vector.tensor_copy(dest, tp4[:])

        # ---- streamed x loads: q=2 on gpsimd (around AG), q=3 on sync ----
        st_tiles = {}

        def issue_stream_load(q, rr, eng):
            row0 = rr * MB + q * 128
            xt = xpool.tile([128, V, D], F32, tag="xraw", name="xraw")
            eng.dma_start(xt[:], x_d[row0 : row0 + 128])
            st_tiles[(q, rr)] = xt

        # own chunk loads on sync first (they gate the AG path; whole
        # chunks — the startup no longer binds, the own0 chain finishes
        # ~33us vs the ~45us trigger deadline)
        own_x = []
        for qc in range(NQ):
            xt = xqpool.tile([128, V, D], F32, tag="xown", name="xown")
            nc.sync.dma_start(xt[:], xq_d[128 * qc : 128 * (qc + 1)])
            own_x.append(xt)

        # streamed loads up front on two queues: q=2 on gpsimd, q=3 on
        # sync (the agin DMAs queue behind gpsimd's 8MB but still trigger
        # well before the ~50us barrier ends)
        for rr in range(NCORES):
            issue_stream_load(2, rr, nc.gpsimd)
        for rr in range(NCORES):
            issue_stream_load(3, rr, nc.sync)

        # ---- own chunks -> QTc, AG for q in {0,1} ----
        for qc in range(NQ):
            ypk, _ = norm_quant(own_x[qc])
            transpose_into(ypk, QTc[qc][:])
            if qc in AGQ:
                nc.gpsimd.dma_start(agin[qc][:], QTc[qc][:])
                nc.gpsimd.collective_compute(
                    "AllGather", mybir.AluOpType.bypass,
                    replica_groups=[list(range(NCORES))],
                    ins=[agin[qc].opt()], outs=[agout[qc].opt()])

        # ---- streamed chunks: local norm -> PE transpose into YTg ----
        last_rs = None
        for q in STQ:
            for rr in range(NCORES):
                ypk, last_rs = norm_quant(st_tiles[(q, rr)])
                g = 2 * q + rr // 4
                c0 = 128 * (rr % 4)
                transpose_into(ypk, YTg[g][:, :, :, c0 : c0 + 128])

        # warm the Ln activation table during scalar's idle window: anchored
        # on the LAST streamed chunk's rs (produced by the final Sqrt), so it
        # provably follows every Square/Sqrt and the finale's Ln skips its
        # ~1.3us table load
        lnwarm = const.tile([128, 1], F32, name="lnwarm")
        nc.scalar.activation(
            lnwarm[:], last_rs[:, 0:1], AF.Ln, scale=-1.0 / 512.0,
            bias=twob[:])

        # ---- AG scatter: agout -> YTg bands. gpsimd ONLY: any engine
        # whose FIFO holds an AG-completion wait must have no later
        # time-critical work (a scalar-queue scatter once stalled the
        # whole normalize pipeline behind an in-flight collective) ----
        for q in AGQ:
            for rr in range(NCORES):
                c0 = 128 * (rr % 4)
                nc.gpsimd.dma_start(
                    YTg[2 * q + rr // 4][:, :, :, c0 : c0 + 128], agout[q][rr])

        # ---- per-group Gram rows + row max ----
        Q8c = [
            QTc[qc].bitcast(F8)[:].rearrange("p v t (m b) -> p v b t m", b=4)
            for qc in range(NQ)
        ]
        for g in (4, 5, 6, 7, 0, 1, 2, 3):
            Y8r = YTg[g].bitcast(F8)[:].rearrange("p v t (k b) -> p v b t k", b=4)
            q_of_g, h_of_g = g // 2, g % 2
            for v in range(V):
                for mc in range(NQ):
                    has_mask = mc == q_of_g
                    acc = accp.tile([128, 512], F32, tag="acc", name="acc")
                    for b in range(4):
                        nc.tensor.matmul(
                            acc[:],
                            Q8c[mc][:, v, b],
                            Y8r[:, v, b],
                            start=(b == 0), stop=(b == 3 and not has_mask),
                            perf_mode=DR)
                    if has_mask:
                        nc.tensor.matmul(
                            acc[:], hselI[:, h_of_g], bandB[:],
                            start=False, stop=True, skip_group_check=True)
                    nc.vector.reduce_max(
                        mxs[:, g, v * NQ + mc : v * NQ + mc + 1], acc[:],
                        axis=mybir.AxisListType.X)

        # ---- finale ----
        # ship the per-row log vector; the -0.5/B * sum is folded into
        # the host-side partial combine. The affine 2 - fm/512 folds into
        # the Ln activation's scale/bias (one less DVE op + cross-engine
        # hop in the final serial chain; the +1e-8 is below ulp of ~1.75)
        fm = const.tile([128, V * NQ], F32, name="fm")
        nc.vector.reduce_max(
            fm[:], mxs.rearrange("p g c -> p c g"), axis=mybir.AxisListType.X)
        lg = const.tile([128, V * NQ], F32, name="lg")
        nc.scalar.activation(
            lg[:], fm[:], AF.Ln, scale=-1.0 / 512.0, bias=twob[:])
        nc.sync.dma_start(out_d, lg[:])

    nc.compile()
    return nc


_CACHED = {}


def _run(x, trace=False):
    x = np.ascontiguousarray(np.asarray(x, dtype=np.float32))
    assert x.shape == (B, V, D), x.shape
    if "nc" not in _CACHED:
        _CACHED["nc"] = build()
    nc = _CACHED["nc"]
    in_maps = []
    for r in range(NCORES):
        band = np.zeros((128, 512), np.float32)
        col0 = (r % 4) * 128
        band[np.arange(128), col0 + np.arange(128)] = MASKV
        hseli = np.zeros((128, 2, 128), np.float32)
        hseli[np.arange(128), r // 4, np.arange(128)] = 1.0
        in_maps.append({
            "x": x,
            "xq": np.ascontiguousarray(x[MB * r : MB * (r + 1)]),
            "bandpat": band,
            "hseli": hseli,
        })
    res = bass_utils.run_bass_kernel_spmd(
        nc, in_maps, core_ids=list(range(NCORES)), trace=trace)
    partials = [
        np.float64(res.results[r]["out"]).sum() * (-0.5 / B)
        for r in range(NCORES)
    ]
    total = np.float32(np.sum(partials))
    return total, res


def kernel(student_global_cls_tokens):
    total, _ = _run(student_global_cls_tokens, trace=False)
    return np.asarray(total, dtype=np.float32)
